# revision 11
# baseline (speedup 1.0000x reference)
"""Self-contained Trainium2 Bass kernel for nn_ActionExpertSelfBlock.

Strategy (8 NeuronCores, SPMD):
  - Sequence-parallel: core c owns tokens [256c, 256c+256) of each batch
    (512 token-columns per core, batches side by side).
  - Feature-major activations [feature, token] everywhere -> no on-device
    transposes.  Attention computes scores^T = k^T.T @ q^T with keys on
    partitions; softmax without max-subtraction (scores provably tiny);
    causal mask applied as per-core 0/1 fp16 multiplicative input data so
    the SPMD graph is identical on every core; softmax denominator via a
    ones-vector matmul; V produced token-major so it feeds PV directly.
  - One device collective: AllGather of rope'd K^T and V (fp16).
  - fp16 compute (1 PE cycle/row like bf16 but 8x finer mantissa),
    fp32 PSUM accumulation.  Output gathered/transposed on host.
"""

import numpy as np

import concourse.bass as bass
import concourse.tile as tile
import concourse.mybir as mybir
from concourse import bacc
from concourse import bass2jax

B, L, D = 2, 2048, 1024
QH, KVH, HD, FFN = 8, 2, 128, 4096
EPS = 1.1920929e-07
N_CORES = 8
TLOC = 256            # tokens per core per batch
T = B * TLOC          # 512 local token columns per core
SCALE = 1.0 / float(np.sqrt(HD))
PAYLOAD = 2 * 128 * T + T * 256   # kT (256x512) + v (512x256) fp16 elements

F16 = mybir.dt.float16
F32 = mybir.dt.float32


def _dram_bcast_ap(scr, parts):
    """Partition-broadcast AP of a [1, N] DRAM AP to [parts, N] (for DMA)."""
    return bass.AP(tensor=scr.tensor, offset=scr.offset,
                   ap=[[0, parts], list(scr.ap[1])])


def _build():
    nc = bacc.Bacc("TRN2", target_bir_lowering=False, debug=False,
                   num_devices=N_CORES)

    xT_e = nc.dram_tensor("xT", [D, T], F32, kind="ExternalInput")
    cos_e = nc.dram_tensor("cosdup", [128, T], F32, kind="ExternalInput")
    sin_e = nc.dram_tensor("sindup", [128, T], F32, kind="ExternalInput")
    mask_e = nc.dram_tensor("mask", [16, 128, 256], F16, kind="ExternalInput")
    wq_e = nc.dram_tensor("wq", [D, QH * HD], F16, kind="ExternalInput")
    wk_e = nc.dram_tensor("wk", [D, KVH * HD], F16, kind="ExternalInput")
    wv_e = nc.dram_tensor("wv", [D, KVH * HD], F16, kind="ExternalInput")
    wo_e = nc.dram_tensor("wo", [QH * HD, D], F16, kind="ExternalInput")
    wg_e = nc.dram_tensor("wg", [D, FFN], F16, kind="ExternalInput")
    wu_e = nc.dram_tensor("wu", [D, FFN], F16, kind="ExternalInput")
    wd_e = nc.dram_tensor("wd", [FFN, D], F16, kind="ExternalInput")
    out_e = nc.dram_tensor("out", [D, T], F32, kind="ExternalOutput")

    with tile.TileContext(nc) as tc:
        _emit(nc, tc, xT_e, cos_e, sin_e, mask_e, wq_e, wk_e, wv_e, wo_e,
              wg_e, wu_e, wd_e, out_e)

    nc.compile()
    return nc


def _emit(nc, tc, xT_e, cos_e, sin_e, mask_e, wq_e, wk_e, wv_e, wo_e,
          wg_e, wu_e, wd_e, out_e):
    from contextlib import ExitStack
    es = ExitStack()          # whole-kernel persistents
    es_x = ExitStack()        # x/qro/ctxn/wo: closed mid-kernel
    with es:
        dram = es.enter_context(tc.tile_pool(name="dram", bufs=1, space="DRAM"))
        in_cc = dram.tile([PAYLOAD], F16)
        out_cc = dram.tile([N_CORES * PAYLOAD], F16, addr_space="Shared")
        kT_cc = in_cc[0:256 * T].rearrange("(r c) -> r c", c=T)      # [256, 512]
        v_cc = in_cc[256 * T:PAYLOAD].rearrange("(r c) -> r c", c=256)  # [512, 256]
        scr_rb = dram.tile([1, T], F32, name="scr_rb", tag="scr_rb", bufs=2)
        scr_dn = dram.tile([1, 1024], F32, name="scr_dn", tag="scr_dn", bufs=2)

        p_ones = es.enter_context(tc.tile_pool(name="p_ones", bufs=2))
        ones1 = p_ones.tile([128, 1], F16, name="ones1", tag="o1")
        nc.vector.memset(ones1, 1.0)
        eps_sb = p_ones.tile([1, 1], F32, name="eps_sb", tag="eps")
        nc.vector.memset(eps_sb, EPS)

        p_x2 = es.enter_context(tc.tile_pool(name="p_x2", bufs=8))
        p_h2 = es.enter_context(tc.tile_pool(name="p_h2", bufs=8))
        p_x = es_x.enter_context(tc.tile_pool(name="p_x", bufs=8))
        p_q = es_x.enter_context(tc.tile_pool(name="p_q", bufs=8))
        p_ctxn = es_x.enter_context(tc.tile_pool(name="p_ctxn", bufs=1))
        p_wo = es_x.enter_context(tc.tile_pool(name="p_wo", bufs=8))

        x_sb = [p_x.tile([128, T], F32, name=f"x{i}", tag="x") for i in range(8)]
        for i in range(8):
            nc.sync.dma_start(out=x_sb[i], in_=xT_e[i * 128:(i + 1) * 128, :])
        ctxn_all = p_ctxn.tile([128, QH * T], F16, name="ctxn_all")

        def rmsnorm(src_tiles, pool_sq, pool_rb, pool_stat, pool_out, out_tag,
                    psum_pool, tagp):
            ss = psum_pool.tile([1, T], F32, name=f"ss_{tagp}", tag="ss")
            for i in range(8):
                sq = pool_sq.tile([128, T], F16, name=f"sq_{tagp}{i}", tag="sq")
                nc.vector.tensor_mul(sq, src_tiles[i], src_tiles[i])
                nc.tensor.matmul(ss, lhsT=ones1, rhs=sq,
                                 start=(i == 0), stop=(i == 7))
            srt = pool_stat.tile([1, T], F32, name=f"srt_{tagp}", tag="st")
            nc.scalar.activation(srt, ss, mybir.ActivationFunctionType.Sqrt,
                                 bias=eps_sb, scale=1.0 / D)
            rinv = pool_stat.tile([1, T], F32, name=f"rinv_{tagp}", tag="st")
            nc.vector.reciprocal_approx_fast(out=rinv, in_=srt)
            nc.sync.dma_start(out=scr_rb, in_=rinv)
            rb = pool_rb.tile([128, T], F32, name=f"rb_{tagp}", tag="rbb")
            nc.sync.dma_start(out=rb, in_=_dram_bcast_ap(scr_rb, 128))
            out_tiles = []
            for i in range(8):
                ht = pool_out.tile([128, T], F16, name=f"h_{tagp}{i}", tag=out_tag)
                nc.vector.tensor_mul(ht, src_tiles[i], rb)
                out_tiles.append(ht)
            return out_tiles

        # =========================================================
        # Phase 1+2: rmsnorm1, K/V + rope + payload DMA + AllGather, Q
        # =========================================================
        with tc.tile_pool(name="p_cs", bufs=2) as p_cs, \
             tc.tile_pool(name="p_sq", bufs=3) as p_sq, \
             tc.tile_pool(name="p_rb", bufs=2) as p_rb, \
             tc.tile_pool(name="p_st", bufs=2) as p_st, \
             tc.tile_pool(name="p_h1", bufs=8) as p_h1, \
             tc.tile_pool(name="p_wkv", bufs=16) as p_wkv, \
             tc.tile_pool(name="p_wq", bufs=8) as p_wq, \
             tc.tile_pool(name="p_m", bufs=6) as p_m, \
             tc.tile_pool(name="p_kro", bufs=2) as p_kro, \
             tc.tile_pool(name="p_vt", bufs=4) as p_vt, \
             tc.tile_pool(name="ps_qkv", bufs=2, space="PSUM") as ps_qkv, \
             tc.tile_pool(name="ps_sm", bufs=2, space="PSUM") as ps_sm:

            cos_sb = p_cs.tile([128, T], F32, name="cos_sb", tag="cs")
            sin_sb = p_cs.tile([128, T], F32, name="sin_sb", tag="cs")
            nc.sync.dma_start(out=cos_sb, in_=cos_e[:, :])
            nc.sync.dma_start(out=sin_sb, in_=sin_e[:, :])

            h_sb = rmsnorm(x_sb, p_sq, p_rb, p_st, p_h1, "h1", ps_sm, "1")

            wk_sb = [p_wkv.tile([128, 256], F16, name=f"wk{i}", tag="wkv")
                     for i in range(8)]
            wv_sb = [p_wkv.tile([128, 256], F16, name=f"wv{i}", tag="wkv")
                     for i in range(8)]
            for i in range(8):
                nc.sync.dma_start(out=wk_sb[i], in_=wk_e[i * 128:(i + 1) * 128, :])
                nc.sync.dma_start(out=wv_sb[i], in_=wv_e[i * 128:(i + 1) * 128, :])

            def rope(ps, dst):
                m1 = p_m.tile([128, T], F16, name="rope_m1", tag="m")
                m2 = p_m.tile([128, T], F16, name="rope_m2", tag="m")
                m2s = p_m.tile([128, T], F16, name="rope_m2s", tag="m")
                nc.vector.tensor_mul(m1, ps, cos_sb)
                nc.vector.tensor_mul(m2, ps, sin_sb)   # sin_sb = [sin; -sin]
                nc.sync.dma_start(out=m2s[0:64, :], in_=m2[64:128, :])
                nc.sync.dma_start(out=m2s[64:128, :], in_=m2[0:64, :])
                nc.vector.tensor_add(dst, m1, m2s)

            for kv in range(KVH):
                ps = ps_qkv.tile([128, T], F32, name=f"ps_k{kv}", tag="qkv")
                for dt in range(8):
                    nc.tensor.matmul(ps, lhsT=wk_sb[dt][:, kv * 128:(kv + 1) * 128],
                                     rhs=h_sb[dt], start=(dt == 0), stop=(dt == 7))
                kro = p_kro.tile([128, T], F16, name=f"kro{kv}", tag="kro")
                rope(ps, kro)
                nc.sync.dma_start(out=kT_cc[kv * 128:(kv + 1) * 128, :], in_=kro)
            for tt in range(4):
                ps = ps_qkv.tile([128, 256], F32, name=f"ps_v{tt}", tag="vps")
                for dt in range(8):
                    nc.tensor.matmul(ps, lhsT=h_sb[dt][:, tt * 128:(tt + 1) * 128],
                                     rhs=wv_sb[dt], start=(dt == 0), stop=(dt == 7))
                vt = p_vt.tile([128, 256], F16, name=f"v{tt}", tag="vt")
                nc.scalar.copy(vt, ps)
                nc.sync.dma_start(out=v_cc[tt * 128:(tt + 1) * 128, :], in_=vt)

            nc.gpsimd.collective_compute(
                "AllGather", mybir.AluOpType.bypass,
                replica_groups=[list(range(N_CORES))],
                ins=[in_cc.opt()], outs=[out_cc.opt()],
            )

            wq_sb = [p_wq.tile([128, QH * HD], F16, name=f"wq{i}", tag="wq")
                     for i in range(8)]
            for i in range(8):
                nc.sync.dma_start(out=wq_sb[i], in_=wq_e[i * 128:(i + 1) * 128, :])
            qro_sb = []
            for f in range(QH):
                ps = ps_qkv.tile([128, T], F32, name=f"ps_q{f}", tag="qkv")
                for dt in range(8):
                    nc.tensor.matmul(ps, lhsT=wq_sb[dt][:, f * 128:(f + 1) * 128],
                                     rhs=h_sb[dt], start=(dt == 0), stop=(dt == 7))
                q = p_q.tile([128, T], F16, name=f"qro{f}", tag="q")
                rope(ps, q)
                qro_sb.append(q)

        # =========================================================
        # Phase 3: attention
        # =========================================================
        def kT_g(src):
            off = src * PAYLOAD
            return out_cc[off: off + 256 * T].rearrange("(r c) -> r c", c=T)

        def v_g(src):
            off = src * PAYLOAD + 256 * T
            return out_cc[off: off + T * 256].rearrange("(r c) -> r c", c=256)

        wo_sb = [p_wo.tile([128, D], F16, name=f"wo{i}", tag="wo") for i in range(8)]
        for i in range(8):
            nc.sync.dma_start(out=wo_sb[i], in_=wo_e[i * 128:(i + 1) * 128, :])

        with tc.tile_pool(name="att_m", bufs=16) as pool_mask, \
             tc.tile_pool(name="att_kts", bufs=2) as pool_kts, \
             tc.tile_pool(name="att_vts", bufs=2) as pool_vts, \
             tc.tile_pool(name="att_p", bufs=7) as pool_pr, \
             tc.tile_pool(name="att_dnsb", bufs=2) as pool_dnsb, \
             tc.tile_pool(name="att_rb", bufs=2) as pool_rbat, \
             tc.tile_pool(name="att_ps", bufs=2, space="PSUM") as ps_s_pool, \
             tc.tile_pool(name="att_ctx", bufs=1, space="PSUM") as ps_ctx_pool, \
             tc.tile_pool(name="att_dn", bufs=1, space="PSUM") as ps_dn_pool:

            mask_sb = [pool_mask.tile([128, 256], F16, name=f"msk{ch}", tag="msk")
                       for ch in range(16)]
            for ch in range(16):
                nc.sync.dma_start(out=mask_sb[ch], in_=mask_e[ch])

            for kv in range(KVH):
                # slab loads: one big DMA per source core for k and for v
                kts = pool_kts.tile([128, N_CORES * T], F16, name=f"kts{kv}", tag="kts")
                vts = pool_vts.tile([128, N_CORES * T], F16, name=f"vts{kv}", tag="vts")
                for src in range(N_CORES):
                    nc.sync.dma_start(
                        out=kts[:, src * T:(src + 1) * T],
                        in_=kT_g(src)[kv * 128:(kv + 1) * 128, :])
                    # v_g(src) is [512 rows, 256]; fold rows into 4 combos of 128
                    vsrc = v_g(src).rearrange("(c r) f -> c r f", r=128)  # [4,128,256]
                    nc.sync.dma_start(
                        out=vts[:, src * T:(src + 1) * T].rearrange(
                            "p (c n) -> p c n", c=4),
                        in_=bass.AP(tensor=vsrc.tensor, offset=vsrc.offset
                                    + kv * 128,
                                    ap=[list(vsrc.ap[1]), list(vsrc.ap[0]),
                                        [1, 128]]))
                for b in range(B):
                    ctx = ps_ctx_pool.tile([128, 1024], F32, name=f"ctx{b}{kv}", tag="ctx")
                    dn = ps_dn_pool.tile([1, 1024], F32, name=f"dn{b}{kv}", tag="dn")
                    def emit_pv(ch, probs):
                        slot = ch % 2
                        vc = vts[:, ((ch // 2) * 4 + b * 2 + slot) * 128:][:, :128]
                        for half in range(2):
                            sl = slice(half * 512, (half + 1) * 512)
                            nc.tensor.matmul(ctx[:, sl], lhsT=vc, rhs=probs[:, sl],
                                             start=(ch == 0), stop=(ch == 15))
                            nc.tensor.matmul(dn[:, sl], lhsT=ones1, rhs=probs[:, sl],
                                             start=(ch == 0), stop=(ch == 15))

                    # software pipeline: PV runs 4 chunks behind QK/exp so the
                    # first PV of this (b, kv) doesn't head-block the in-order
                    # PE stream on the previous iteration's ctx normalize.
                    pq = []
                    for ch in range(16):
                        src, slot = ch // 2, ch % 2
                        kc = kts[:, src * T + b * 256 + slot * 128:][:, :128]
                        ps = ps_s_pool.tile([128, 1024], F32, name=f"ps{b}{kv}{ch}", tag="ps_s")
                        for h4 in range(4):
                            nc.tensor.matmul(
                                ps[:, h4 * 256:(h4 + 1) * 256], lhsT=kc,
                                rhs=qro_sb[kv * 4 + h4][:, b * 256:(b + 1) * 256],
                                start=True, stop=True)
                        probs = pool_pr.tile([128, 1024], F16, name=f"pr{b}{kv}{ch}", tag="pr")
                        nc.scalar.activation(probs, ps,
                                             mybir.ActivationFunctionType.Exp,
                                             scale=SCALE)
                        mview = bass.AP(tensor=mask_sb[ch].tensor,
                                        offset=mask_sb[ch].offset,
                                        ap=[list(mask_sb[ch].ap[0]), [0, 4],
                                            list(mask_sb[ch].ap[1])])
                        pview = probs.rearrange("p (h q) -> p h q", h=4)
                        nc.vector.tensor_mul(pview, pview, mview)
                        pq.append((ch, probs))
                        if len(pq) > 4:
                            emit_pv(*pq.pop(0))
                    for item in pq:
                        emit_pv(*item)
                    dn_sb = pool_dnsb.tile([1, 1024], F32, name=f"dnsb{b}{kv}", tag="dnsb")
                    nc.scalar.copy(dn_sb, dn)
                    rec = pool_dnsb.tile([1, 1024], F32, name=f"rec{b}{kv}", tag="dnsb")
                    nc.vector.reciprocal_approx_fast(out=rec, in_=dn_sb)
                    nc.sync.dma_start(out=scr_dn, in_=rec)
                    rb = pool_rbat.tile([128, 1024], F32, name=f"rb{b}{kv}", tag="rbat")
                    nc.sync.dma_start(out=rb, in_=_dram_bcast_ap(scr_dn, 128))
                    cview = bass.AP(
                        tensor=ctxn_all.tensor,
                        offset=ctxn_all.offset + kv * 4 * 512 + b * 256,
                        ap=[list(ctxn_all.ap[0]), [512, 4], [1, 256]])
                    nc.vector.tensor_mul(
                        cview,
                        ctx.rearrange("p (h q) -> p h q", h=4),
                        rb.rearrange("p (h q) -> p h q", h=4))

        # =========================================================
        # Phase 4: o_proj + residual + rmsnorm2
        # =========================================================
        with tc.tile_pool(name="ph4_sq", bufs=3) as p_sq4, \
             tc.tile_pool(name="ph4_rb", bufs=2) as p_rb4, \
             tc.tile_pool(name="ph4_st", bufs=2) as p_st4, \
             tc.tile_pool(name="ph4_ps", bufs=2, space="PSUM") as ps_o_pool, \
             tc.tile_pool(name="ph4_ps2", bufs=2, space="PSUM") as ps_sm2:
            x2_sb = []
            for m in range(8):
                ps = ps_o_pool.tile([128, T], F32, name=f"ps_o{m}", tag="po")
                for f in range(8):
                    nc.tensor.matmul(ps, lhsT=wo_sb[f][:, m * 128:(m + 1) * 128],
                                     rhs=ctxn_all[:, f * T:(f + 1) * T],
                                     start=(f == 0), stop=(f == 7))
                x2 = p_x2.tile([128, T], F32, name=f"x2_{m}", tag="x2")
                nc.vector.tensor_add(x2, ps, x_sb[m])
                x2_sb.append(x2)
            h2_sb = rmsnorm(x2_sb, p_sq4, p_rb4, p_st4, p_h2, "h2k", ps_sm2, "2")
        es_x.close()   # frees x, qro, ctxn, wo

        # =========================================================
        # Phase 5: FFN
        # =========================================================
        with tc.tile_pool(name="p_hm", bufs=32) as p_hm, \
             tc.tile_pool(name="ph5_wg", bufs=16) as pool_wg, \
             tc.tile_pool(name="ph5_wu", bufs=16) as pool_wu, \
             tc.tile_pool(name="ph5_sg", bufs=4) as pool_sg, \
             tc.tile_pool(name="ph5_ps", bufs=2, space="PSUM") as ps_f_pool:
            hm_sb = [p_hm.tile([128, T], F16, name=f"hm{fo}", tag="hm")
                     for fo in range(32)]
            for fc in range(8):
                wg_sl = [pool_wg.tile([128, 512], F16, name=f"wg{fc}_{i}", tag="wgsl")
                         for i in range(8)]
                wu_sl = [pool_wu.tile([128, 512], F16, name=f"wu{fc}_{i}", tag="wusl")
                         for i in range(8)]
                for i in range(8):
                    nc.sync.dma_start(
                        out=wg_sl[i],
                        in_=wg_e[i * 128:(i + 1) * 128, fc * 512:(fc + 1) * 512])
                    nc.sync.dma_start(
                        out=wu_sl[i],
                        in_=wu_e[i * 128:(i + 1) * 128, fc * 512:(fc + 1) * 512])
                for j in range(4):
                    fo = fc * 4 + j
                    psg = ps_f_pool.tile([128, T], F32, name=f"psg{fo}", tag="pg")
                    psu = ps_f_pool.tile([128, T], F32, name=f"psu{fo}", tag="pu")
                    for dt in range(8):
                        nc.tensor.matmul(psg, lhsT=wg_sl[dt][:, j * 128:(j + 1) * 128],
                                         rhs=h2_sb[dt], start=(dt == 0), stop=(dt == 7))
                    for dt in range(8):
                        nc.tensor.matmul(psu, lhsT=wu_sl[dt][:, j * 128:(j + 1) * 128],
                                         rhs=h2_sb[dt], start=(dt == 0), stop=(dt == 7))
                    sg = pool_sg.tile([128, T], F16, name=f"sg{fo}", tag="sg")
                    nc.scalar.activation(sg, psg,
                                         mybir.ActivationFunctionType.Silu)
                    nc.vector.tensor_mul(hm_sb[fo], psu, sg)

            with tc.tile_pool(name="ph5b_w", bufs=6) as pool_wd, \
                 tc.tile_pool(name="ph5b_ps", bufs=4, space="PSUM") as ps_d_pool, \
                 tc.tile_pool(name="ph5b_o", bufs=4) as pool_out:
                for grp in range(2):
                    ps_d = [ps_d_pool.tile([128, T], F32, name=f"psd{grp}{mm}", tag="pd")
                            for mm in range(4)]
                    for fo in range(32):
                        wd_sl = pool_wd.tile([128, D], F16, name=f"wd{grp}_{fo}", tag="wd")
                        nc.sync.dma_start(out=wd_sl,
                                          in_=wd_e[fo * 128:(fo + 1) * 128, :])
                        for mm in range(4):
                            m = grp * 4 + mm
                            nc.tensor.matmul(ps_d[mm],
                                             lhsT=wd_sl[:, m * 128:(m + 1) * 128],
                                             rhs=hm_sb[fo],
                                             start=(fo == 0), stop=(fo == 31))
                    for mm in range(4):
                        m = grp * 4 + mm
                        ot = pool_out.tile([128, T], F32, name=f"ot{m}", tag="ot")
                        nc.vector.tensor_add(ot, ps_d[mm], x2_sb[m])
                        nc.sync.dma_start(out=out_e[m * 128:(m + 1) * 128, :], in_=ot)


_CACHE = {}


def _get_nc():
    if "nc" not in _CACHE:
        _CACHE["nc"] = _build()
    return _CACHE["nc"]


def _host_prep(x, ln1_w, wq, wk, wv, wo, ln2_w, wg, wu, wd):
    f32 = np.float32
    f16 = np.float16
    x = np.asarray(x, f32)
    ln1 = np.asarray(ln1_w, f32)
    ln2 = np.asarray(ln2_w, f32)
    wq_h = (ln1[:, None] * np.asarray(wq, f32)).astype(f16)
    wk_h = (ln1[:, None] * np.asarray(wk, f32)).astype(f16)
    wv_h = (ln1[:, None] * np.asarray(wv, f32)).astype(f16)
    wo_h = np.asarray(wo, f32).astype(f16)
    wg_h = (ln2[:, None] * np.asarray(wg, f32)).astype(f16)
    wu_h = (ln2[:, None] * np.asarray(wu, f32)).astype(f16)
    wd_h = np.asarray(wd, f32).astype(f16)

    d2 = HD // 2
    ts = 10000.0 ** ((2.0 / HD) * np.arange(d2, dtype=f32))
    pos = np.arange(L, dtype=f32)
    rad = pos[:, None] / ts[None, :]
    cos = np.cos(rad).astype(f32)
    sin = np.sin(rad).astype(f32)

    in_maps = []
    for c in range(N_CORES):
        sl = slice(TLOC * c, TLOC * (c + 1))
        xT = np.ascontiguousarray(
            np.concatenate([x[b, sl, :].T for b in range(B)], axis=1))
        cs = np.concatenate([cos[sl].T, cos[sl].T], axis=0)
        sn = np.concatenate([sin[sl].T, -sin[sl].T], axis=0)
        cosdup = np.ascontiguousarray(np.concatenate([cs, cs], axis=1))
        sindup = np.ascontiguousarray(np.concatenate([sn, sn], axis=1))
        kg = np.arange(128)[:, None]
        mask = np.zeros((16, 128, 2, 128), dtype=f16)
        for ch in range(16):
            for blk in range(2):
                gq = TLOC * c + 128 * blk + np.arange(128)[None, :]
                mask[ch, :, blk, :] = (128 * ch + kg <= gq).astype(f16)
        in_maps.append(dict(
            xT=xT, cosdup=cosdup, sindup=sindup,
            mask=np.ascontiguousarray(mask.reshape(16, 128, 256)),
            wq=wq_h, wk=wk_h, wv=wv_h, wo=wo_h, wg=wg_h, wu=wu_h, wd=wd_h))
    return in_maps


def kernel(x, ln1_w, wq, wk, wv, wo, ln2_w, wg, wu, wd):
    nc = _get_nc()
    in_maps = _host_prep(x, ln1_w, wq, wk, wv, wo, ln2_w, wg, wu, wd)
    results = bass2jax.run_bass_via_pjrt(nc, in_maps, n_cores=N_CORES)
    out = np.empty((B, L, D), dtype=np.float32)
    for c in range(N_CORES):
        o = results[c]["out"]          # [D, T]
        for b in range(B):
            out[b, TLOC * c:TLOC * (c + 1), :] = o[:, b * TLOC:(b + 1) * TLOC].T
    return out


# revision 13
# speedup vs baseline: 1.3154x; 1.3154x over previous
"""Self-contained Trainium2 Bass kernel for nn_ActionExpertSelfBlock.

Strategy (8 NeuronCores, SPMD):
  - Sequence-parallel: core c owns tokens [256c, 256c+256) of each batch
    (512 token-columns per core, batches side by side).
  - Feature-major activations [feature, token] everywhere -> no on-device
    transposes.  Attention computes scores^T = k^T.T @ q^T with keys on
    partitions; softmax without max-subtraction (scores provably tiny);
    causal mask applied as per-core 0/1 fp16 multiplicative input data so
    the SPMD graph is identical on every core; softmax denominator via a
    ones-vector matmul; V produced token-major so it feeds PV directly.
  - One device collective: AllGather of rope'd K^T and V (fp16).
  - fp16 compute (1 PE cycle/row like bf16 but 8x finer mantissa),
    fp32 PSUM accumulation.  Output gathered/transposed on host.
"""

import numpy as np

import concourse.bass as bass
import concourse.tile as tile
import concourse.mybir as mybir
from concourse import bacc
from concourse import bass2jax

B, L, D = 2, 2048, 1024
QH, KVH, HD, FFN = 8, 2, 128, 4096
EPS = 1.1920929e-07
N_CORES = 8
TLOC = 256            # tokens per core per batch
T = B * TLOC          # 512 local token columns per core
SCALE = 1.0 / float(np.sqrt(HD))
PAYLOAD = 2 * 128 * T + T * 256   # kT (256x512) + v (512x256) fp16 elements

F16 = mybir.dt.float16
F32 = mybir.dt.float32


def _dram_bcast_ap(scr, parts):
    """Partition-broadcast AP of a [1, N] DRAM AP to [parts, N] (for DMA)."""
    return bass.AP(tensor=scr.tensor, offset=scr.offset,
                   ap=[[0, parts], list(scr.ap[1])])


def _build():
    nc = bacc.Bacc("TRN2", target_bir_lowering=False, debug=False,
                   num_devices=N_CORES)

    xT_e = nc.dram_tensor("xT", [D, T], F32, kind="ExternalInput")
    cos_e = nc.dram_tensor("cosdup", [128, T], F32, kind="ExternalInput")
    sin_e = nc.dram_tensor("sindup", [128, T], F32, kind="ExternalInput")
    mask_e = nc.dram_tensor("mask", [16, 128, 256], F16, kind="ExternalInput")
    wq_e = nc.dram_tensor("wq", [D, QH * HD], F16, kind="ExternalInput")
    wk_e = nc.dram_tensor("wk", [D, KVH * HD], F16, kind="ExternalInput")
    wv_e = nc.dram_tensor("wv", [D, KVH * HD], F16, kind="ExternalInput")
    wo_e = nc.dram_tensor("wo", [QH * HD, D], F16, kind="ExternalInput")
    wg_e = nc.dram_tensor("wg", [D, FFN], F16, kind="ExternalInput")
    wu_e = nc.dram_tensor("wu", [D, FFN], F16, kind="ExternalInput")
    wd_e = nc.dram_tensor("wd", [FFN, D], F16, kind="ExternalInput")
    out_e = nc.dram_tensor("out", [D, T], F32, kind="ExternalOutput")

    with tile.TileContext(nc) as tc:
        _emit(nc, tc, xT_e, cos_e, sin_e, mask_e, wq_e, wk_e, wv_e, wo_e,
              wg_e, wu_e, wd_e, out_e)

    nc.compile()
    return nc


def _emit(nc, tc, xT_e, cos_e, sin_e, mask_e, wq_e, wk_e, wv_e, wo_e,
          wg_e, wu_e, wd_e, out_e):
    from contextlib import ExitStack
    es = ExitStack()          # whole-kernel persistents
    es_x = ExitStack()        # x/qro/ctxn/wo: closed mid-kernel
    with es:
        dram = es.enter_context(tc.tile_pool(name="dram", bufs=1, space="DRAM"))
        in_cc = dram.tile([PAYLOAD], F16)
        out_cc = dram.tile([N_CORES * PAYLOAD], F16, addr_space="Shared")
        kT_cc = in_cc[0:256 * T].rearrange("(r c) -> r c", c=T)      # [256, 512]
        v_cc = in_cc[256 * T:PAYLOAD].rearrange("(r c) -> r c", c=256)  # [512, 256]
        scr_rb = dram.tile([1, T], F32, name="scr_rb", tag="scr_rb", bufs=2)
        scr_dn = dram.tile([1, 1024], F32, name="scr_dn", tag="scr_dn", bufs=2)

        p_ones = es.enter_context(tc.tile_pool(name="p_ones", bufs=2))
        ones1 = p_ones.tile([128, 1], F16, name="ones1", tag="o1")
        nc.vector.memset(ones1, 1.0)
        eps_sb = p_ones.tile([1, 1], F32, name="eps_sb", tag="eps")
        nc.vector.memset(eps_sb, EPS)

        p_x2 = es.enter_context(tc.tile_pool(name="p_x2", bufs=8))
        p_h2 = es.enter_context(tc.tile_pool(name="p_h2", bufs=8))
        p_x = es_x.enter_context(tc.tile_pool(name="p_x", bufs=8))
        p_q = es_x.enter_context(tc.tile_pool(name="p_q", bufs=8))
        p_ctxn = es_x.enter_context(tc.tile_pool(name="p_ctxn", bufs=1))
        p_wo = es_x.enter_context(tc.tile_pool(name="p_wo", bufs=8))

        x_sb = [p_x.tile([128, T], F32, name=f"x{i}", tag="x") for i in range(8)]
        for i in range(8):
            nc.sync.dma_start(out=x_sb[i], in_=xT_e[i * 128:(i + 1) * 128, :])
        ctxn_all = p_ctxn.tile([128, QH * T], F16, name="ctxn_all")

        def rmsnorm(src_tiles, pool_sq, pool_rb, pool_stat, pool_out, out_tag,
                    psum_pool, tagp):
            ss = psum_pool.tile([1, T], F32, name=f"ss_{tagp}", tag="ss")
            for i in range(8):
                sq = pool_sq.tile([128, T], F16, name=f"sq_{tagp}{i}", tag="sq")
                nc.vector.tensor_mul(sq, src_tiles[i], src_tiles[i])
                nc.tensor.matmul(ss, lhsT=ones1, rhs=sq,
                                 start=(i == 0), stop=(i == 7))
            srt = pool_stat.tile([1, T], F32, name=f"srt_{tagp}", tag="st")
            nc.scalar.activation(srt, ss, mybir.ActivationFunctionType.Sqrt,
                                 bias=eps_sb, scale=1.0 / D)
            rinv = pool_stat.tile([1, T], F32, name=f"rinv_{tagp}", tag="st")
            nc.vector.reciprocal_approx_fast(out=rinv, in_=srt)
            nc.sync.dma_start(out=scr_rb, in_=rinv)
            rb = pool_rb.tile([128, T], F32, name=f"rb_{tagp}", tag="rbb")
            nc.sync.dma_start(out=rb, in_=_dram_bcast_ap(scr_rb, 128))
            out_tiles = []
            for i in range(8):
                ht = pool_out.tile([128, T], F16, name=f"h_{tagp}{i}", tag=out_tag)
                nc.vector.tensor_mul(ht, src_tiles[i], rb)
                out_tiles.append(ht)
            return out_tiles

        # =========================================================
        # Phase 1+2: rmsnorm1, K/V + rope + payload DMA + AllGather, Q
        # =========================================================
        with tc.tile_pool(name="p_cs", bufs=2) as p_cs, \
             tc.tile_pool(name="p_sq", bufs=3) as p_sq, \
             tc.tile_pool(name="p_rb", bufs=2) as p_rb, \
             tc.tile_pool(name="p_st", bufs=2) as p_st, \
             tc.tile_pool(name="p_h1", bufs=8) as p_h1, \
             tc.tile_pool(name="p_wkv", bufs=16) as p_wkv, \
             tc.tile_pool(name="p_wq", bufs=8) as p_wq, \
             tc.tile_pool(name="p_m", bufs=6) as p_m, \
             tc.tile_pool(name="p_kro", bufs=2) as p_kro, \
             tc.tile_pool(name="p_vt", bufs=4) as p_vt, \
             tc.tile_pool(name="ps_qkv", bufs=2, space="PSUM") as ps_qkv, \
             tc.tile_pool(name="ps_sm", bufs=2, space="PSUM") as ps_sm:

            cos_sb = p_cs.tile([128, T], F32, name="cos_sb", tag="cs")
            sin_sb = p_cs.tile([128, T], F32, name="sin_sb", tag="cs")
            nc.sync.dma_start(out=cos_sb, in_=cos_e[:, :])
            nc.sync.dma_start(out=sin_sb, in_=sin_e[:, :])

            h_sb = rmsnorm(x_sb, p_sq, p_rb, p_st, p_h1, "h1", ps_sm, "1")

            wk_sb = [p_wkv.tile([128, 256], F16, name=f"wk{i}", tag="wkv")
                     for i in range(8)]
            wv_sb = [p_wkv.tile([128, 256], F16, name=f"wv{i}", tag="wkv")
                     for i in range(8)]
            for i in range(8):
                nc.sync.dma_start(out=wk_sb[i], in_=wk_e[i * 128:(i + 1) * 128, :])
                nc.sync.dma_start(out=wv_sb[i], in_=wv_e[i * 128:(i + 1) * 128, :])

            def rope(ps, dst):
                m1 = p_m.tile([128, T], F16, name="rope_m1", tag="m")
                m2 = p_m.tile([128, T], F16, name="rope_m2", tag="m")
                m2s = p_m.tile([128, T], F16, name="rope_m2s", tag="m")
                nc.vector.tensor_mul(m1, ps, cos_sb)
                nc.vector.tensor_mul(m2, ps, sin_sb)   # sin_sb = [sin; -sin]
                nc.sync.dma_start(out=m2s[0:64, :], in_=m2[64:128, :])
                nc.sync.dma_start(out=m2s[64:128, :], in_=m2[0:64, :])
                nc.vector.tensor_add(dst, m1, m2s)

            for kv in range(KVH):
                ps = ps_qkv.tile([128, T], F32, name=f"ps_k{kv}", tag="qkv")
                for dt in range(8):
                    nc.tensor.matmul(ps, lhsT=wk_sb[dt][:, kv * 128:(kv + 1) * 128],
                                     rhs=h_sb[dt], start=(dt == 0), stop=(dt == 7))
                kro = p_kro.tile([128, T], F16, name=f"kro{kv}", tag="kro")
                rope(ps, kro)
                nc.sync.dma_start(out=kT_cc[kv * 128:(kv + 1) * 128, :], in_=kro)
            for tt in range(4):
                ps = ps_qkv.tile([128, 256], F32, name=f"ps_v{tt}", tag="vps")
                for dt in range(8):
                    nc.tensor.matmul(ps, lhsT=h_sb[dt][:, tt * 128:(tt + 1) * 128],
                                     rhs=wv_sb[dt], start=(dt == 0), stop=(dt == 7))
                vt = p_vt.tile([128, 256], F16, name=f"v{tt}", tag="vt")
                nc.scalar.copy(vt, ps)
                nc.sync.dma_start(out=v_cc[tt * 128:(tt + 1) * 128, :], in_=vt)

            nc.gpsimd.collective_compute(
                "AllGather", mybir.AluOpType.bypass,
                replica_groups=[list(range(N_CORES))],
                ins=[in_cc.opt()], outs=[out_cc.opt()],
            )

            wq_sb = [p_wq.tile([128, QH * HD], F16, name=f"wq{i}", tag="wq")
                     for i in range(8)]
            for i in range(8):
                nc.sync.dma_start(out=wq_sb[i], in_=wq_e[i * 128:(i + 1) * 128, :])
            qro_sb = []
            for f in range(QH):
                ps = ps_qkv.tile([128, T], F32, name=f"ps_q{f}", tag="qkv")
                for dt in range(8):
                    nc.tensor.matmul(ps, lhsT=wq_sb[dt][:, f * 128:(f + 1) * 128],
                                     rhs=h_sb[dt], start=(dt == 0), stop=(dt == 7))
                q = p_q.tile([128, T], F16, name=f"qro{f}", tag="q")
                rope(ps, q)
                qro_sb.append(q)

        # =========================================================
        # Phase 3: attention
        # =========================================================
        def kT_g(src):
            off = src * PAYLOAD
            return out_cc[off: off + 256 * T].rearrange("(r c) -> r c", c=T)

        def v_g(src):
            off = src * PAYLOAD + 256 * T
            return out_cc[off: off + T * 256].rearrange("(r c) -> r c", c=256)

        wo_sb = [p_wo.tile([128, D], F16, name=f"wo{i}", tag="wo") for i in range(8)]
        for i in range(8):
            nc.sync.dma_start(out=wo_sb[i], in_=wo_e[i * 128:(i + 1) * 128, :])

        with tc.tile_pool(name="att_m", bufs=16) as pool_mask, \
             tc.tile_pool(name="att_kts", bufs=2) as pool_kts, \
             tc.tile_pool(name="att_vts", bufs=2) as pool_vts, \
             tc.tile_pool(name="att_p", bufs=7) as pool_pr, \
             tc.tile_pool(name="att_dnsb", bufs=2) as pool_dnsb, \
             tc.tile_pool(name="att_rb", bufs=2) as pool_rbat, \
             tc.tile_pool(name="att_ps", bufs=2, space="PSUM") as ps_s_pool, \
             tc.tile_pool(name="att_ctx", bufs=1, space="PSUM") as ps_ctx_pool, \
             tc.tile_pool(name="att_dn", bufs=1, space="PSUM") as ps_dn_pool:

            mask_sb = [pool_mask.tile([128, 256], F16, name=f"msk{ch}", tag="msk")
                       for ch in range(16)]
            for ch in range(16):
                nc.sync.dma_start(out=mask_sb[ch], in_=mask_e[ch])

            for kv in range(KVH):
                # slab loads: one big DMA per source core for k and for v
                kts = pool_kts.tile([128, N_CORES * T], F16, name=f"kts{kv}", tag="kts")
                vts = pool_vts.tile([128, N_CORES * T], F16, name=f"vts{kv}", tag="vts")
                for src in range(N_CORES):
                    nc.sync.dma_start(
                        out=kts[:, src * T:(src + 1) * T],
                        in_=kT_g(src)[kv * 128:(kv + 1) * 128, :])
                    # v_g(src) is [512 rows, 256]; fold rows into 4 combos of 128
                    vsrc = v_g(src).rearrange("(c r) f -> c r f", r=128)  # [4,128,256]
                    nc.sync.dma_start(
                        out=vts[:, src * T:(src + 1) * T].rearrange(
                            "p (c n) -> p c n", c=4),
                        in_=bass.AP(tensor=vsrc.tensor, offset=vsrc.offset
                                    + kv * 128,
                                    ap=[list(vsrc.ap[1]), list(vsrc.ap[0]),
                                        [1, 128]]))
                for b in range(B):
                    ctx = ps_ctx_pool.tile([128, 1024], F32, name=f"ctx{b}{kv}", tag="ctx")
                    dn = ps_dn_pool.tile([1, 1024], F32, name=f"dn{b}{kv}", tag="dn")
                    def emit_pv(ch, probs):
                        slot = ch % 2
                        vc = vts[:, ((ch // 2) * 4 + b * 2 + slot) * 128:][:, :128]
                        for half in range(2):
                            sl = slice(half * 512, (half + 1) * 512)
                            nc.tensor.matmul(ctx[:, sl], lhsT=vc, rhs=probs[:, sl],
                                             start=(ch == 0), stop=(ch == 15))
                            nc.tensor.matmul(dn[:, sl], lhsT=ones1, rhs=probs[:, sl],
                                             start=(ch == 0), stop=(ch == 15))

                    # software pipeline: PV runs 4 chunks behind QK/exp so the
                    # first PV of this (b, kv) doesn't head-block the in-order
                    # PE stream on the previous iteration's ctx normalize.
                    pq = []
                    for ch in range(16):
                        src, slot = ch // 2, ch % 2
                        kc = kts[:, src * T + b * 256 + slot * 128:][:, :128]
                        ps = ps_s_pool.tile([128, 1024], F32, name=f"ps{b}{kv}{ch}", tag="ps_s")
                        for h4 in range(4):
                            nc.tensor.matmul(
                                ps[:, h4 * 256:(h4 + 1) * 256], lhsT=kc,
                                rhs=qro_sb[kv * 4 + h4][:, b * 256:(b + 1) * 256],
                                start=True, stop=True)
                        probs = pool_pr.tile([128, 1024], F16, name=f"pr{b}{kv}{ch}", tag="pr")
                        nc.scalar.activation(probs, ps,
                                             mybir.ActivationFunctionType.Exp,
                                             scale=SCALE)
                        mview = bass.AP(tensor=mask_sb[ch].tensor,
                                        offset=mask_sb[ch].offset,
                                        ap=[list(mask_sb[ch].ap[0]), [0, 4],
                                            list(mask_sb[ch].ap[1])])
                        pview = probs.rearrange("p (h q) -> p h q", h=4)
                        nc.vector.tensor_mul(pview, pview, mview)
                        pq.append((ch, probs))
                        if len(pq) > 4:
                            emit_pv(*pq.pop(0))
                    for item in pq:
                        emit_pv(*item)
                    dn_sb = pool_dnsb.tile([1, 1024], F32, name=f"dnsb{b}{kv}", tag="dnsb")
                    nc.scalar.copy(dn_sb, dn)
                    rec = pool_dnsb.tile([1, 1024], F32, name=f"rec{b}{kv}", tag="dnsb")
                    nc.vector.reciprocal_approx_fast(out=rec, in_=dn_sb)
                    nc.sync.dma_start(out=scr_dn, in_=rec)
                    rb = pool_rbat.tile([128, 1024], F32, name=f"rb{b}{kv}", tag="rbat")
                    nc.sync.dma_start(out=rb, in_=_dram_bcast_ap(scr_dn, 128))
                    cview = bass.AP(
                        tensor=ctxn_all.tensor,
                        offset=ctxn_all.offset + kv * 4 * 512 + b * 256,
                        ap=[list(ctxn_all.ap[0]), [512, 4], [1, 256]])
                    nc.vector.tensor_mul(
                        cview,
                        ctx.rearrange("p (h q) -> p h q", h=4),
                        rb.rearrange("p (h q) -> p h q", h=4))

        # =========================================================
        # Phase 4: o_proj + residual + rmsnorm2
        # =========================================================
        with tc.tile_pool(name="ph4_sq", bufs=3) as p_sq4, \
             tc.tile_pool(name="ph4_rb", bufs=2) as p_rb4, \
             tc.tile_pool(name="ph4_st", bufs=2) as p_st4, \
             tc.tile_pool(name="ph4_ps", bufs=2, space="PSUM") as ps_o_pool, \
             tc.tile_pool(name="ph4_ps2", bufs=2, space="PSUM") as ps_sm2:
            x2_sb = []
            for m in range(8):
                ps = ps_o_pool.tile([128, T], F32, name=f"ps_o{m}", tag="po")
                for f in range(8):
                    nc.tensor.matmul(ps, lhsT=wo_sb[f][:, m * 128:(m + 1) * 128],
                                     rhs=ctxn_all[:, f * T:(f + 1) * T],
                                     start=(f == 0), stop=(f == 7))
                x2 = p_x2.tile([128, T], F32, name=f"x2_{m}", tag="x2")
                nc.vector.tensor_add(x2, ps, x_sb[m])
                x2_sb.append(x2)
            h2_sb = rmsnorm(x2_sb, p_sq4, p_rb4, p_st4, p_h2, "h2k", ps_sm2, "2")
        es_x.close()   # frees x, qro, ctxn, wo

        # =========================================================
        # Phase 5: FFN
        # =========================================================
        with tc.tile_pool(name="p_hm", bufs=32) as p_hm, \
             tc.tile_pool(name="ph5_wg", bufs=16) as pool_wg, \
             tc.tile_pool(name="ph5_wu", bufs=16) as pool_wu, \
             tc.tile_pool(name="ph5_sg", bufs=4) as pool_sg, \
             tc.tile_pool(name="ph5_ps", bufs=2, space="PSUM") as ps_f_pool:
            hm_sb = [p_hm.tile([128, T], F16, name=f"hm{fo}", tag="hm")
                     for fo in range(32)]
            for fc in range(8):
                wg_sl = [pool_wg.tile([128, 512], F16, name=f"wg{fc}_{i}", tag="wgsl")
                         for i in range(8)]
                wu_sl = [pool_wu.tile([128, 512], F16, name=f"wu{fc}_{i}", tag="wusl")
                         for i in range(8)]
                for i in range(8):
                    nc.sync.dma_start(
                        out=wg_sl[i],
                        in_=wg_e[i * 128:(i + 1) * 128, fc * 512:(fc + 1) * 512])
                    nc.sync.dma_start(
                        out=wu_sl[i],
                        in_=wu_e[i * 128:(i + 1) * 128, fc * 512:(fc + 1) * 512])
                for j in range(4):
                    fo = fc * 4 + j
                    psg = ps_f_pool.tile([128, T], F32, name=f"psg{fo}", tag="pg")
                    psu = ps_f_pool.tile([128, T], F32, name=f"psu{fo}", tag="pu")
                    for dt in range(8):
                        nc.tensor.matmul(psg, lhsT=wg_sl[dt][:, j * 128:(j + 1) * 128],
                                         rhs=h2_sb[dt], start=(dt == 0), stop=(dt == 7))
                    for dt in range(8):
                        nc.tensor.matmul(psu, lhsT=wu_sl[dt][:, j * 128:(j + 1) * 128],
                                         rhs=h2_sb[dt], start=(dt == 0), stop=(dt == 7))
                    sg = pool_sg.tile([128, T], F16, name=f"sg{fo}", tag="sg")
                    nc.scalar.activation(sg, psg,
                                         mybir.ActivationFunctionType.Silu)
                    nc.vector.tensor_mul(hm_sb[fo], psu, sg)

            with tc.tile_pool(name="ph5b_w", bufs=6) as pool_wd, \
                 tc.tile_pool(name="ph5b_ps", bufs=4, space="PSUM") as ps_d_pool, \
                 tc.tile_pool(name="ph5b_o", bufs=4) as pool_out:
                for grp in range(2):
                    ps_d = [ps_d_pool.tile([128, T], F32, name=f"psd{grp}{mm}", tag="pd")
                            for mm in range(4)]
                    for fo in range(32):
                        wd_sl = pool_wd.tile([128, D], F16, name=f"wd{grp}_{fo}", tag="wd")
                        nc.sync.dma_start(out=wd_sl,
                                          in_=wd_e[fo * 128:(fo + 1) * 128, :])
                        for mm in range(4):
                            m = grp * 4 + mm
                            nc.tensor.matmul(ps_d[mm],
                                             lhsT=wd_sl[:, m * 128:(m + 1) * 128],
                                             rhs=hm_sb[fo],
                                             start=(fo == 0), stop=(fo == 31))
                    for mm in range(4):
                        m = grp * 4 + mm
                        ot = pool_out.tile([128, T], F32, name=f"ot{m}", tag="ot")
                        nc.vector.tensor_add(ot, ps_d[mm], x2_sb[m])
                        nc.sync.dma_start(out=out_e[m * 128:(m + 1) * 128, :], in_=ot)


_CACHE = {}


def _get_nc():
    if "nc" not in _CACHE:
        _CACHE["nc"] = _build()
    return _CACHE["nc"]


def _host_prep(x, ln1_w, wq, wk, wv, wo, ln2_w, wg, wu, wd):
    f32 = np.float32
    f16 = np.float16
    x = np.asarray(x, f32)
    ln1 = np.asarray(ln1_w, f32)
    ln2 = np.asarray(ln2_w, f32)
    wq_h = (ln1[:, None] * np.asarray(wq, f32)).astype(f16)
    wk_h = (ln1[:, None] * np.asarray(wk, f32)).astype(f16)
    wv_h = (ln1[:, None] * np.asarray(wv, f32)).astype(f16)
    wo_h = np.asarray(wo, f32).astype(f16)
    wg_h = (ln2[:, None] * np.asarray(wg, f32)).astype(f16)
    wu_h = (ln2[:, None] * np.asarray(wu, f32)).astype(f16)
    wd_h = np.asarray(wd, f32).astype(f16)

    d2 = HD // 2
    ts = 10000.0 ** ((2.0 / HD) * np.arange(d2, dtype=f32))
    pos = np.arange(L, dtype=f32)
    rad = pos[:, None] / ts[None, :]
    cos = np.cos(rad).astype(f32)
    sin = np.sin(rad).astype(f32)

    in_maps = []
    for c in range(N_CORES):
        sl = slice(TLOC * c, TLOC * (c + 1))
        xT = np.ascontiguousarray(
            np.concatenate([x[b, sl, :].T for b in range(B)], axis=1))
        cs = np.concatenate([cos[sl].T, cos[sl].T], axis=0)
        sn = np.concatenate([sin[sl].T, -sin[sl].T], axis=0)
        cosdup = np.ascontiguousarray(np.concatenate([cs, cs], axis=1))
        sindup = np.ascontiguousarray(np.concatenate([sn, sn], axis=1))
        kg = np.arange(128)[:, None]
        mask = np.zeros((16, 128, 2, 128), dtype=f16)
        for ch in range(16):
            for blk in range(2):
                gq = TLOC * c + 128 * blk + np.arange(128)[None, :]
                mask[ch, :, blk, :] = (128 * ch + kg <= gq).astype(f16)
        in_maps.append(dict(
            xT=xT, cosdup=cosdup, sindup=sindup,
            mask=np.ascontiguousarray(mask.reshape(16, 128, 256)),
            wq=wq_h, wk=wk_h, wv=wv_h, wo=wo_h, wg=wg_h, wu=wu_h, wd=wd_h))
    return in_maps


def kernel(x, ln1_w, wq, wk, wv, wo, ln2_w, wg, wu, wd):
    nc = _get_nc()
    in_maps = _host_prep(x, ln1_w, wq, wk, wv, wo, ln2_w, wg, wu, wd)
    results = bass2jax.run_bass_via_pjrt(nc, in_maps, n_cores=N_CORES)
    out = np.empty((B, L, D), dtype=np.float32)
    for c in range(N_CORES):
        o = results[c]["out"]          # [D, T]
        for b in range(B):
            out[b, TLOC * c:TLOC * (c + 1), :] = o[:, b * TLOC:(b + 1) * TLOC].T
    return out
